# revision 3
# baseline (speedup 1.0000x reference)
import os
import time
import numpy as np
import jax
import jax.numpy as jnp
from jax.sharding import Mesh, PartitionSpec as P, NamedSharding
from jax.experimental.shard_map import shard_map

# nn_GatedFusionBlockCustom: B=8, S=2048, H=256, NH=8 heads.
# Wire-optimized for the axon tunnel (~74ms RTT, ~13-20ms/MB, h2d compressed,
# d2h NOT compressed, single host CPU):
#   - gating MLPs + audio branch (LN1 -> aproj -> outproj, folded into one
#     256x256 GEMM) run on host in f32; only z = gm*attn_output + video is
#     shipped, int8 per-batch scale.
#   - all device weights are f32, uploaded ONCE and cached on device
#     (validated by byte-compare of the param tensors each call).
#   - work is split into GROUPS of cores; per group the host computes z,
#     quantizes, and issues put+dispatch+fetch, so host compute of later
#     groups overlaps the wire of earlier ones.
#   - the device returns delta = final - z_dequant quantized to 6 bits
#     (packed 4 vals -> 3 bytes; d2h does not compress so packing pays);
#     the host reconstructs final = z_exact + delta, which cancels the
#     direct z-quant error (only the indirect error through LN/FFN/attn
#     survives).
#   - gate broadcast outputs are returned as stride-0 views (no 32MB copies).
B, S, H, NH = 8, 2048, 256, 8
DH = H // NH

def _parse_groups():
    spec = os.environ.get('KGROUPS', '4,2,2')
    sizes = [int(x) for x in spec.split(',')]
    assert sum(sizes) == 8
    groups, i = [], 0
    for s in sizes:
        groups.append(list(range(i, i + s)))
        i += s
    return groups


GROUPS = _parse_groups()

MM_KEYS = ['ffn1_w1', 'ffn1_w2', 'ffn2_w1', 'ffn2_w2', 'attn_in_w', 'attn_out_w']
SMALL_KEYS = ['ffn1_b1', 'ffn1_b2', 'ffn2_b1', 'ffn2_b2', 'attn_in_b',
              'attn_out_b', 'n2_g', 'n2_b', 'n3_g', 'n3_b', 'n4_g', 'n4_b']
HOST_KEYS = ['g_mha_w1', 'g_mha_b1', 'g_mha_w2', 'g_mha_b2',
             'g_ffn_w1', 'g_ffn_b1', 'g_ffn_w2', 'g_ffn_b2',
             'aproj_w', 'aproj_b', 'outproj_w', 'outproj_b', 'n1_g', 'n1_b']
PARAM_KEYS = MM_KEYS + SMALL_KEYS + HOST_KEYS

_MM_SHAPES = {'ffn1_w1': (4 * H, H), 'ffn1_w2': (H, 4 * H),
              'ffn2_w1': (4 * H, H), 'ffn2_w2': (H, 4 * H),
              'attn_in_w': (3 * H, H), 'attn_out_w': (H, H)}
_MM_SIZES = [int(np.prod(_MM_SHAPES[k])) for k in MM_KEYS]
_MM_OFFS = np.cumsum([0] + _MM_SIZES)
_SMALL_SHAPES = {k: (4 * H,) if k in ('ffn1_b1', 'ffn2_b1') else
                 ((3 * H,) if k == 'attn_in_b' else (H,)) for k in SMALL_KEYS}
_SM_SIZES = [int(np.prod(_SMALL_SHAPES[k])) for k in SMALL_KEYS]
_SM_OFFS = np.cumsum([0] + _SM_SIZES) + _MM_OFFS[-1]
_WTOT = int(_SM_OFFS[-1])                 # 1315840 f32 elements
_WPAD = _WTOT + ((-_WTOT) % 16)
_WB = _WPAD * 4                            # full weight buffer bytes
_WSH = _WB // 8                            # bytes per core for upload

_ZB = S * H
_ZNB = _ZB + 8                             # u8 z + f32 [zscale, gf]

DBITS = int(os.environ.get('KDBITS', '6'))  # delta quant bits (6 -> pack 4:3)
_DQ = 2 ** (DBITS - 1) - 1                 # 31
_PB = S * H * 3 // 4 if DBITS == 6 else S * H  # packed delta bytes per core
_ONB = _PB + 4                             # + f32 dscale

_devs = jax.devices()[:8]
_mesh8 = Mesh(np.asarray(_devs), ("core",))
_S8 = NamedSharding(_mesh8, P("core"))


def _bitcast_f32(u8):
    return jax.lax.bitcast_convert_type(u8.reshape(-1, 4), jnp.float32).reshape(-1)


def _prep_block(wrow):
    # wrow [1, _WSH] u8 -> per-core full weight buffer [1, _WB]
    full = jax.lax.all_gather(wrow[0], "core", axis=0, tiled=True)
    return full[None]


def _core_block(wrow, zrow):
    w = _bitcast_f32(wrow[0])
    ws = [w[_MM_OFFS[i]:_MM_OFFS[i + 1]].reshape(_MM_SHAPES[MM_KEYS[i]][::-1])
          for i in range(len(MM_KEYS))]
    f1w1t, f1w2t, f2w1t, f2w2t, attn_in_wt, attn_out_wt = ws
    sm = [w[_SM_OFFS[i]:_SM_OFFS[i + 1]] for i in range(len(SMALL_KEYS))]
    (f1b1, f1b2, f2b1, f2b2, attn_in_b, attn_out_b,
     n2g, n2b, n3g, n3b, n4g, n4b) = sm

    zq = zrow[0][:_ZB]
    tail = _bitcast_f32(zrow[0][_ZB:])
    zscale, gfb = tail[0], tail[1]
    z = (zq.reshape(S, H).astype(jnp.float32) - 128.0) * zscale

    def ln(x, g, b, eps=1e-5):
        m = x.mean(-1, keepdims=True)
        v = ((x - m) ** 2).mean(-1, keepdims=True)
        return (x - m) * jax.lax.rsqrt(v + eps) * g + b

    h1 = jnp.maximum(ln(z, n2g, n2b) @ f1w1t + f1b1, 0.0) @ f1w2t + f1b2
    z_bar = gfb * h1 + z
    x3 = ln(z_bar, n3g, n3b)
    qkv = x3 @ attn_in_wt + attn_in_b
    q, k, v = jnp.split(qkv, 3, axis=-1)
    q = q.reshape(S, NH, DH)
    k = k.reshape(S, NH, DH)
    v = v.reshape(S, NH, DH)
    scores = jnp.einsum('qhd,khd->hqk', q, k,
                        preferred_element_type=jnp.float32) * (DH ** -0.5)
    attn = jax.nn.softmax(scores, axis=-1)
    ctx = jnp.einsum('hqk,khd->qhd', attn, v,
                     preferred_element_type=jnp.float32).reshape(S, H)
    refined = ctx @ attn_out_wt + attn_out_b + z_bar
    final = (jnp.maximum(ln(refined, n4g, n4b) @ f2w1t + f2b1, 0.0)
             @ f2w2t + f2b2 + refined)

    delta = final - z
    dscale = jnp.maximum(jnp.max(jnp.abs(delta)), 1e-20) / _DQ
    qd = jnp.clip(jnp.round(delta / dscale) + (_DQ + 1.0), 0.0,
                  2.0 * _DQ + 1.0).astype(jnp.uint8)
    if DBITS == 6:
        r = qd.reshape(-1, 4)
        b0 = r[:, 0] | (r[:, 1] << 6)
        b1 = (r[:, 1] >> 2) | (r[:, 2] << 4)
        b2 = (r[:, 2] >> 4) | (r[:, 3] << 2)
        packed = jnp.stack([b0, b1, b2], axis=1).reshape(-1)
    else:
        packed = qd.reshape(-1)
    out = jnp.concatenate([
        packed, jax.lax.bitcast_convert_type(dscale[None], jnp.uint8).reshape(-1)])
    return out[None]


_cache = None  # (params_copy, d_w per group, wgc, bgc, gate_ws, fns)


def _build_cache(inputs):
    global _cache
    f = lambda k: np.asarray(inputs[k], np.float32)
    wflat = np.zeros(_WPAD, np.float32)
    for i, k in enumerate(MM_KEYS):
        wflat[_MM_OFFS[i]:_MM_OFFS[i + 1]] = \
            np.ascontiguousarray(f(k).T).reshape(-1)
    for i, k in enumerate(SMALL_KEYS):
        wflat[_SM_OFFS[i]:_SM_OFFS[i + 1]] = f(k)
    wbuf = wflat.view(np.uint8).reshape(8, _WSH)
    d8 = jax.device_put(wbuf, _S8)
    prep = jax.jit(shard_map(_prep_block, mesh=_mesh8, in_specs=(P("core"),),
                             out_specs=P("core"), check_rep=False))
    wall = prep(d8)
    wall.block_until_ready()
    shmap = {sh.device: sh.data for sh in wall.addressable_shards}

    d_w, fns, shardings = [], [], []
    for g in GROUPS:
        mesh_g = Mesh(np.asarray([_devs[i] for i in g]), ("c",))
        s_g = NamedSharding(mesh_g, P("c"))
        dw = jax.make_array_from_single_device_arrays(
            (len(g), _WB), s_g, [shmap[_devs[i]] for i in g])
        fn = jax.jit(shard_map(_core_block, mesh=mesh_g,
                               in_specs=(P("c"), P("c")),
                               out_specs=P("c"), check_rep=False))
        d_w.append(dw)
        fns.append(fn)
        shardings.append(s_g)

    # host-side folded audio-branch weights
    wc = f('outproj_w') @ f('aproj_w')
    bc = f('outproj_w') @ f('aproj_b') + f('outproj_b')
    wgc_t = np.ascontiguousarray((wc * f('n1_g')[None, :]).T)  # [H, H] for d @ wgc_t
    bgc = wc @ f('n1_b') + bc
    gate_ws = {k: f(k) for k in HOST_KEYS[:8]}
    params_copy = {k: np.array(inputs[k], np.float32, copy=True) for k in PARAM_KEYS}
    _cache = dict(params=params_copy, d_w=d_w, fns=fns, shardings=shardings,
                  wgc_t=wgc_t, bgc=bgc, gate_ws=gate_ws)


def _params_match(inputs):
    if _cache is None:
        return False
    p = _cache['params']
    for k in PARAM_KEYS:
        if not np.array_equal(np.asarray(inputs[k]), p[k]):
            return False
    return True


_TMP = np.empty((S, H), np.float32)
_TMP2 = np.empty((S, H), np.float32)
_UNPK = np.empty((S * H // 4, 4), np.uint8)
_LUTB = np.arange(2 ** DBITS, dtype=np.float32) - (_DQ + 1.0)


_PROF = bool(os.environ.get('KPROF'))
_IMPLICIT = os.environ.get('KIMPLICIT', '1') != '0'


def kernel(**inputs):
    t0 = time.perf_counter()
    lg = (lambda s: print(f"  [{s}] {(time.perf_counter()-t0)*1e3:.1f}ms",
                          flush=True)) if _PROF else (lambda s: None)
    if not _params_match(inputs):
        _build_cache(inputs)
        # pre-warm every per-call code path (relay, dispatch caches, numpy
        # lazy inits) so the first post-cold call runs at steady state
        _run(inputs, _cache, lambda s: None)
        _run(inputs, _cache, lambda s: None)
    c = _cache
    lg("cache ok")
    return _run(inputs, c, lg)


def _run(inputs, c, lg):
    video = np.asarray(inputs['video_feat'], np.float32)
    audio = np.asarray(inputs['audio_feat'], np.float32)
    gw = c['gate_ws']
    wgc_t, bgc = c['wgc_t'], c['bgc']

    final = np.empty((B, S, H), np.float32)
    gm = np.empty((B, 1), np.float32)
    gf = np.empty((B, 1), np.float32)
    outs = []
    for gi, g in enumerate(GROUPS):
        zpack = np.empty((len(g), _ZNB), np.uint8)
        for row, b in enumerate(g):
            audio_b = audio[b]
            video_b = video[b]
            # gates for this batch
            joint = np.concatenate([video_b.mean(0), audio_b.mean(0)])[None]
            h = np.maximum(joint @ gw['g_mha_w1'].T + gw['g_mha_b1'], 0.0)
            gm[b] = np.tanh(h @ gw['g_mha_w2'].T + gw['g_mha_b2'])
            h = np.maximum(joint @ gw['g_ffn_w1'].T + gw['g_ffn_b1'], 0.0)
            gf[b] = np.tanh(h @ gw['g_ffn_w2'].T + gw['g_ffn_b2'])
            # z = gm * outproj(aproj(LN1(audio_b))) + video_b (folded GEMM)
            mu = audio_b.mean(-1, keepdims=True)
            d = np.subtract(audio_b, mu, out=_TMP)
            var = np.einsum('sh,sh->s', d, d, dtype=np.float32) * np.float32(1.0 / H)
            rs = (1.0 / np.sqrt(var + np.float32(1e-5))) * gm[b, 0]
            d *= rs[:, None]
            zb = final[b]                 # reuse output storage for exact z
            np.dot(d, wgc_t, out=zb)
            zb += gm[b, 0] * bgc[None, :]
            zb += video_b
            zs = np.float32(max(max(zb.max(), -zb.min()) / 127.0, 1e-20))
            t = np.multiply(zb, np.float32(1.0 / zs), out=_TMP2)
            t += np.float32(128.5)
            np.copyto(zpack[row, :_ZB].reshape(S, H), t, casting='unsafe')
            zpack[row, _ZB:] = np.array([zs, gf[b, 0]],
                                        np.float32).view(np.uint8)
        lg(f"g{gi} host done")
        if _IMPLICIT:
            y = c['fns'][gi](c['d_w'][gi], zpack)
        else:
            d_z = jax.device_put(zpack, c['shardings'][gi])
            y = c['fns'][gi](c['d_w'][gi], d_z)
        y.copy_to_host_async()
        outs.append(y)
        lg(f"g{gi} issued")

    for gi, g in enumerate(GROUPS):
        shards = sorted(outs[gi].addressable_shards,
                        key=lambda sh: sh.index[0].start)
        for row, sh in enumerate(shards):
            b = g[row]
            rowdata = np.asarray(sh.data).reshape(_ONB)
            dscale = rowdata[_PB:].copy().view(np.float32)[0]
            if DBITS == 6:
                w3 = rowdata[:_PB].reshape(-1, 3)
                b0, b1, b2 = w3[:, 0], w3[:, 1], w3[:, 2]
                q = _UNPK
                q[:, 0] = b0 & 63
                q[:, 1] = (b0 >> 6) | ((b1 & 15) << 2)
                q[:, 2] = (b1 >> 4) | ((b2 & 3) << 4)
                q[:, 3] = b2 >> 2
                qflat = q.reshape(-1)
            else:
                qflat = rowdata[:_PB]
            lut = _LUTB * dscale
            final[b] += np.take(lut, qflat).reshape(S, H)
            lg(f"b{b} reconstructed")

    gm_full = np.broadcast_to(gm[:, :, None], (B, S, H))
    gf_full = np.broadcast_to(gf[:, :, None], (B, S, H))
    return final, gm_full, gf_full
